# revision 10
# baseline (speedup 1.0000x reference)
"""Trainium2 Bass kernel for nn_MultiHeadAttention (B=8192, D=1024, 16 heads
used only via the softmax scale 1/8).

Strategy (8 NeuronCores, row-sharded with chunked K/V all-gather):
  - Rows (batch axis) of the attention output are sharded: core c owns rows
    [c*1024, (c+1)*1024).
  - Each core computes Q^T, K^T, V only for its OWN 1024 rows; K^T (fp16)
    and V (bf16) shards are exchanged with four chunked HBM AllGather
    collectives fired per 512-row half in K1,V1,K2,V2 order, so the first
    attention blocks are fed while later chunks are still in flight.
    (fp16 over bf16: same 1 cyc/row PE rate and 2-byte comm, but 10
    mantissa bits keep the energy near-f32 accurate; fp8/bf16 energy
    fails/erodes the 2e-2 gate. Chunks stay <=8 MB gathered — larger
    f32r chunks drop AllGather to ~95 GB/s.)
  - All weight DMAs + PE transposes run up front, before any collective
    traffic competes for the DMA engines.
  - Attention runs in a transposed-energy ("E^T") layout so no probability
    transpose is ever needed:
        E^T[j, i] = sum_o K^T[o, j] * Q^T[o, i]      (fp16 operands)
        P^T = exp(E^T * 0.125)                        (no max subtraction)
        out_unnorm[i, o] = sum_j P^T[j, i] * V[j, o]  (bf16)
        s[i] = sum_j P^T[j, i]                        (matmul vs ones)
        out = out_unnorm / s + bv
  - Phase 2 walks j-blocks in gather-arrival order: chunk 0 of every rank,
    then chunk 1.
"""

import sys

sys.path.insert(0, "/opt/trn_rl_repo")

import numpy as np

import concourse.bass as bass  # noqa: F401
import concourse.tile as tile
from concourse import bacc, mybir
from concourse.bass_utils import run_bass_kernel_spmd
from concourse.masks import make_identity

B = 8192
D = 1024
P = 128
NCORES = 8
R = B // NCORES  # 1024 rows per core
JBLK = 512  # j-block (keys/values) streamed per iteration
NH = R // JBLK  # 2 local row-halves (= gather chunks)
DO = D // P  # 8 feature chunks of 128
IC = R // P  # 8 row chunks of 128 per core
F32 = mybir.dt.float32
F32R = mybir.dt.float32r
BF16 = mybir.dt.bfloat16
F16 = mybir.dt.float16
AF = mybir.ActivationFunctionType
ALU = mybir.AluOpType
SCALE = 0.125  # 1/sqrt(head_dim=64)
RG = [list(range(NCORES))]


def build_program():
    nc = bacc.Bacc(
        "TRN2", target_bir_lowering=False, debug=False, num_devices=NCORES
    )
    # f32r is bit-identical to f32 (np.float32 feeds it); declaring the
    # matmul-bound inputs as f32r keeps their DMAs cast-free
    x_loc = nc.dram_tensor("x_loc", [R, D], F32R, kind="ExternalInput").ap()
    w_q = nc.dram_tensor("Wq", [D, D], F32R, kind="ExternalInput").ap()
    w_k = nc.dram_tensor("Wk", [D, D], F32R, kind="ExternalInput").ap()
    w_v = nc.dram_tensor("Wv", [D, D], F32R, kind="ExternalInput").ap()
    b_q = nc.dram_tensor("bq", [D], F32, kind="ExternalInput").ap()
    b_k = nc.dram_tensor("bk", [D], F32, kind="ExternalInput").ap()
    b_v = nc.dram_tensor("bv", [D], F32, kind="ExternalInput").ap()
    out_loc = nc.dram_tensor("out_loc", [R, D], F32, kind="ExternalOutput").ap()

    # local K^T/V shard chunks and their all-gathered forms (collective
    # in/out must be Internal DRAM; outputs Shared for the HBM-HBM path)
    kt_loc = nc.dram_tensor("kt_loc", [NH, DO, P, JBLK], F16)
    v_loc = nc.dram_tensor("v_loc", [NH, JBLK // P, P, D], BF16)
    kt_all = [
        nc.dram_tensor(f"kt_all{h}", [NCORES, DO, P, JBLK], F16, addr_space="Shared")
        for h in range(NH)
    ]
    v_all = [
        nc.dram_tensor(
            f"v_all{h}", [NCORES, JBLK // P, P, D], BF16, addr_space="Shared"
        )
        for h in range(NH)
    ]

    with tile.TileContext(nc) as tc:
        _body(
            nc, tc, x_loc, w_q, w_k, w_v, b_q, b_k, b_v, out_loc,
            kt_loc, v_loc, kt_all, v_all,
        )
    nc.compile()
    return nc


def _transpose_block(nc, tp_psum, identity, src_sb, dst, dd, col_off, drain_eng):
    """PE-transpose one [128, 128] block of src into dst[:, dd, col_off:+128],
    draining the PSUM through the given engine (scalar or vector)."""
    tp = tp_psum.tile([P, P], F32R, tag="tp")
    nc.tensor.transpose(tp, src_sb[:, dd * P : (dd + 1) * P], identity)
    if drain_eng == "scalar":
        nc.scalar.activation(dst[:, dd, col_off : col_off + P], tp, AF.Identity)
    else:
        nc.vector.tensor_copy(out=dst[:, dd, col_off : col_off + P], in_=tp)


def _body(nc, tc, x_loc, w_q, w_k, w_v, b_q, b_k, b_v, out_loc,
          kt_loc, v_loc, kt_all, v_all):
    from contextlib import ExitStack

    outer = ExitStack()
    outer.__enter__()
    # ---- persistent pools (whole kernel) ----
    const_pool = outer.enter_context(tc.tile_pool(name="const", bufs=1))
    identity_f32 = const_pool.tile([P, P], F32)
    make_identity(nc, identity_f32)
    identity = const_pool.tile([P, P], F32R)  # f32r for 1.5 cyc/row transposes
    nc.vector.tensor_copy(out=identity, in_=identity_f32)
    ones_f32 = const_pool.tile([P, 2], F32)
    nc.vector.memset(ones_f32, 1.0)
    ones = const_pool.tile([P, 2], BF16)
    nc.vector.tensor_copy(out=ones, in_=ones_f32)
    bq_sb = const_pool.tile([P, DO], F32)
    nc.sync.dma_start(bq_sb, b_q.rearrange("(oo p) -> p oo", p=P))
    bk_sb = const_pool.tile([P, DO], F32)
    nc.sync.dma_start(bk_sb, b_k.rearrange("(oo p) -> p oo", p=P))
    ones_row = const_pool.tile([1, P], F32)
    nc.vector.memset(ones_row, 1.0)
    # broadcast bv across all 128 partitions with a K=1 matmul
    bv_bc = const_pool.tile([P, D], F32)
    nc.sync.dma_start(bv_bc[0:1, :], b_v[None, :])
    with tc.tile_pool(name="bv_psum", bufs=2, space="PSUM") as bvp:
        for oh in range(2):
            pt = bvp.tile([P, 512], F32, tag="bvp")
            nc.tensor.matmul(
                pt,
                ones_row,
                bv_bc[0:1, oh * 512 : (oh + 1) * 512],
                start=True,
                stop=True,
            )
            nc.vector.tensor_copy(out=bv_bc[:, oh * 512 : (oh + 1) * 512], in_=pt)

    qt_pool = outer.enter_context(tc.tile_pool(name="qt", bufs=1))
    qt = qt_pool.tile([P, DO, R], F16)  # Q^T fp16: [o_in, o_out, i] (2 MB)

    sums_pool = outer.enter_context(tc.tile_pool(name="sums", bufs=1))
    sums_acc = sums_pool.tile([P, 2 * IC], F32)  # per-row exp-sums (even cols)
    rsum = sums_pool.tile([P, 2 * IC], F32)

    # =========================================================
    # Phase 1: weights+x^T up front, then K/V per row-half with
    # gathers fired K1,V1,K2,V2; Q^T last (overlaps the gathers)
    # =========================================================
    with ExitStack() as p1:
        wt_pool = p1.enter_context(tc.tile_pool(name="wt", bufs=1))
        wqt = wt_pool.tile([P, DO, D], F32R)  # W^T: [d_in, d_out, o] (4 MB)
        wkt = wt_pool.tile([P, DO, D], F32R)
        wvt = wt_pool.tile([P, DO, D], BF16)

        xt_pool = p1.enter_context(tc.tile_pool(name="xt", bufs=1))
        xt = xt_pool.tile([P, DO, R], F32R)  # x^T local (4 MB)
        xtb = xt_pool.tile([P, DO, R], BF16)  # bf16 copy for V matmul (2 MB)

        row_pool = p1.enter_context(tc.tile_pool(name="rows", bufs=3))
        st_pool = p1.enter_context(tc.tile_pool(name="stage", bufs=2))
        tp_psum = p1.enter_context(tc.tile_pool(name="tp_ps", bufs=2, space="PSUM"))
        mm_psum = p1.enter_context(tc.tile_pool(name="mm_ps", bufs=4, space="PSUM"))

        # -- all weight + x transposes first (their DMAs beat the AG traffic)
        for wt_sb, w_dram in ((wkt, w_k), (wvt, w_v), (wqt, w_q)):
            for oo in range(DO):
                wrow = row_pool.tile([P, D], F32R, tag="row")
                nc.sync.dma_start(wrow, w_dram[oo * P : (oo + 1) * P, :])
                for dd in range(DO):
                    _transpose_block(
                        nc, tp_psum, identity, wrow, wt_sb, dd, oo * P,
                        "scalar" if dd % 2 else "vector",
                    )
        for jj in range(IC):
            xrow = row_pool.tile([P, D], F32R, tag="row")
            nc.sync.dma_start(xrow, x_loc[jj * P : (jj + 1) * P, :])
            for dd in range(DO):
                _transpose_block(
                    nc, tp_psum, identity, xrow, xt, dd, jj * P,
                    "scalar" if dd % 2 else "vector",
                )
        nc.vector.tensor_copy(out=xtb, in_=xt)

        # -- per row-half: K^T shard then V shard, gathers K1,V1,K2,V2 --
        for ih in range(NH):
            for oo in range(DO):
                pk = mm_psum.tile([P, JBLK], F32, tag="mm")
                for dd in range(DO):
                    nc.tensor.matmul(
                        pk,
                        (wkt[:, dd, oo * P : (oo + 1) * P]),
                        (xt[:, dd, ih * JBLK : (ih + 1) * JBLK]),
                        start=(dd == 0),
                        stop=(dd == DO - 1),
                    )
                kst = st_pool.tile([P, JBLK], F16, tag="kst")
                nc.scalar.activation(
                    kst, pk, AF.Identity, bias=bk_sb[:, oo : oo + 1]
                )
                nc.sync.dma_start(kt_loc[ih, oo], kst)
            nc.gpsimd.collective_compute(
                "AllGather",
                mybir.AluOpType.bypass,
                replica_groups=RG,
                ins=[kt_loc[ih].opt()],
                outs=[kt_all[ih][:, :, :, :].opt()],
            )
            for jj in range(JBLK // P):
                vst = st_pool.tile([P, D], BF16, tag="vst")
                pv_h = [
                    mm_psum.tile([P, JBLK], F32, tag="mm", name="pv")
                    for _ in range(2)
                ]
                for dd in range(DO):
                    for oh in range(2):
                        nc.tensor.matmul(
                            pv_h[oh],
                            (xtb[:, dd, (ih * 4 + jj) * P : (ih * 4 + jj + 1) * P]),
                            (wvt[:, dd, oh * 512 : (oh + 1) * 512]),
                            start=(dd == 0),
                            stop=(dd == DO - 1),
                        )
                for oh in range(2):
                    nc.vector.tensor_copy(
                        out=vst[:, oh * 512 : (oh + 1) * 512], in_=pv_h[oh]
                    )
                nc.sync.dma_start(v_loc[ih, jj], vst)
            nc.gpsimd.collective_compute(
                "AllGather",
                mybir.AluOpType.bypass,
                replica_groups=RG,
                ins=[v_loc[ih].opt()],
                outs=[v_all[ih][:, :, :, :].opt()],
            )

        # -- Q^T (f32r, biased) — overlaps the in-flight gathers --
        for ih in range(NH):
            for oo in range(DO):
                pq = mm_psum.tile([P, JBLK], F32, tag="mm")
                for dd in range(DO):
                    nc.tensor.matmul(
                        pq,
                        (wqt[:, dd, oo * P : (oo + 1) * P]),
                        (xt[:, dd, ih * JBLK : (ih + 1) * JBLK]),
                        start=(dd == 0),
                        stop=(dd == DO - 1),
                    )
                nc.scalar.activation(
                    qt[:, oo, ih * JBLK : (ih + 1) * JBLK],
                    pq,
                    AF.Identity,
                    bias=bq_sb[:, oo : oo + 1],
                )

    # =========================================================
    # Phase 2: streamed attention in E^T layout, blocks walked in
    # gather-arrival order (chunk 0 of every rank, then chunk 1)
    # =========================================================
    with ExitStack() as p2:
        oa_pool = p2.enter_context(tc.tile_pool(name="oacc", bufs=1))
        outacc = oa_pool.tile([P, IC, D], F32)  # 4 MB

        kt_pool = p2.enter_context(tc.tile_pool(name="ktb", bufs=3))
        v_pool = p2.enter_context(tc.tile_pool(name="vtb", bufs=3))
        pt_pool = p2.enter_context(tc.tile_pool(name="ptb", bufs=3))
        e_psum = p2.enter_context(tc.tile_pool(name="e_ps", bufs=4, space="PSUM"))
        o_psum = p2.enter_context(tc.tile_pool(name="o_ps", bufs=3, space="PSUM"))
        s_psum = p2.enter_context(tc.tile_pool(name="s_ps", bufs=1, space="PSUM"))

        first = True
        for ch in range(NH):
            for g in range(NCORES):
                ktb = kt_pool.tile([P, DO, JBLK], F16, tag="ktb")
                for oo in range(DO):
                    nc.sync.dma_start(ktb[:, oo, :], kt_all[ch][g, oo])
                vtb = v_pool.tile([P, JBLK // P, D], BF16, tag="vtb")
                nc.sync.dma_start(
                    vtb, v_all[ch][g].rearrange("jj p o -> p jj o")
                )
                # unnormalized probabilities P^T for this j-block: [j, i]
                ptb = pt_pool.tile([P, JBLK // P, R], BF16, tag="ptb")
                for jj in range(JBLK // P):
                    pe_h = [
                        e_psum.tile([P, JBLK], F32, tag="pe", name="pe")
                        for _ in range(R // JBLK)
                    ]
                    for oo in range(DO):
                        for ih in range(R // JBLK):
                            nc.tensor.matmul(
                                pe_h[ih],
                                (ktb[:, oo, jj * P : (jj + 1) * P]),
                                (qt[:, oo, ih * JBLK : (ih + 1) * JBLK]),
                                start=(oo == 0),
                                stop=(oo == DO - 1),
                            )
                    for ih in range(R // JBLK):
                        nc.scalar.activation(
                            ptb[:, jj, ih * JBLK : (ih + 1) * JBLK],
                            pe_h[ih],
                            AF.Exp,
                            scale=SCALE,
                        )
                # out_unnorm += P^T.T @ V; exp-sums matmul shares each
                # stationary ptb tile (3 streams per weight load)
                ps = s_psum.tile([P, 2 * IC], F32, tag="ps")
                for ic in range(IC):
                    po_h = [
                        o_psum.tile([P, 512], F32, tag="po", name="po")
                        for _ in range(2)
                    ]
                    for jj in range(JBLK // P):
                        for oh in range(2):
                            nc.tensor.matmul(
                                po_h[oh],
                                (ptb[:, jj, ic * P : (ic + 1) * P]),
                                (vtb[:, jj, oh * 512 : (oh + 1) * 512]),
                                start=(jj == 0),
                                stop=(jj == JBLK // P - 1),
                            )
                        nc.tensor.matmul(
                            ps[:, 2 * ic : 2 * ic + 2],
                            (ptb[:, jj, ic * P : (ic + 1) * P]),
                            (ones),
                            start=(ic == 0 and jj == 0),
                            stop=(ic == IC - 1 and jj == JBLK // P - 1),
                        )
                    for oh in range(2):
                        dst = outacc[:, ic, oh * 512 : (oh + 1) * 512]
                        if first:
                            nc.vector.tensor_copy(out=dst, in_=po_h[oh])
                        else:
                            nc.vector.tensor_tensor(dst, po_h[oh], dst, ALU.add)
                if first:
                    nc.vector.tensor_copy(out=sums_acc, in_=ps)
                else:
                    nc.vector.tensor_tensor(sums_acc, ps, sums_acc, ALU.add)
                first = False

        # ---- epilogue: normalize, add bv, write out ----
        nc.vector.reciprocal(rsum, sums_acc)
        fin_pool = p2.enter_context(tc.tile_pool(name="fin", bufs=2))
        for ic in range(IC):
            ofin = fin_pool.tile([P, D], F32, tag="ofin")
            nc.vector.tensor_scalar_mul(ofin, outacc[:, ic, :], rsum[:, 2 * ic : 2 * ic + 1])
            nc.gpsimd.tensor_tensor(ofin, ofin, bv_bc, ALU.add)
            nc.sync.dma_start(out_loc[ic * P : (ic + 1) * P, :], ofin)

    outer.close()


_NC_CACHE = None


def _get_program():
    global _NC_CACHE
    if _NC_CACHE is None:
        _NC_CACHE = build_program()
    return _NC_CACHE


def _run(inputs, trace=False):
    nc = _get_program()
    x = np.ascontiguousarray(np.asarray(inputs["x"], dtype=np.float32))
    common = {
        k: np.ascontiguousarray(np.asarray(inputs[k], dtype=np.float32))
        for k in ("Wq", "Wk", "Wv", "bq", "bk", "bv")
    }
    in_maps = [
        {"x_loc": np.ascontiguousarray(x[c * R : (c + 1) * R]), **common}
        for c in range(NCORES)
    ]
    res = run_bass_kernel_spmd(
        nc, in_maps, core_ids=list(range(NCORES)), trace=trace
    )
    out = np.concatenate([res.results[c]["out_loc"] for c in range(NCORES)], axis=0)
    return out.reshape(B, D, 1).astype(np.float32), res


def kernel(**inputs):
    out, _ = _run(inputs, trace=False)
    return out


# revision 11
# speedup vs baseline: 1.0076x; 1.0076x over previous
"""Trainium2 Bass kernel for nn_MultiHeadAttention (B=8192, D=1024, 16 heads
used only via the softmax scale 1/8).

Strategy (8 NeuronCores, row-sharded with chunked K/V all-gather):
  - Rows (batch axis) of the attention output are sharded: core c owns rows
    [c*1024, (c+1)*1024).
  - Each core computes Q^T, K^T, V only for its OWN 1024 rows; K^T (fp16)
    and V (bf16) shards are exchanged with four chunked HBM AllGather
    collectives fired per 512-row half in K1,V1,K2,V2 order, so the first
    attention blocks are fed while later chunks are still in flight.
    (fp16 over bf16: same 1 cyc/row PE rate and 2-byte comm, but 10
    mantissa bits keep the energy near-f32 accurate; fp8/bf16 energy
    fails/erodes the 2e-2 gate. Chunks stay <=8 MB gathered — larger
    f32r chunks drop AllGather to ~95 GB/s.)
  - All weight DMAs + PE transposes run up front, before any collective
    traffic competes for the DMA engines.
  - Attention runs in a transposed-energy ("E^T") layout so no probability
    transpose is ever needed:
        E^T[j, i] = sum_o K^T[o, j] * Q^T[o, i]      (fp16 operands)
        P^T = exp(E^T * 0.125)                        (no max subtraction)
        out_unnorm[i, o] = sum_j P^T[j, i] * V[j, o]  (bf16)
        s[i] = sum_j P^T[j, i]                        (matmul vs ones)
        out = out_unnorm / s + bv
  - Phase 2 walks j-blocks in gather-arrival order: chunk 0 of every rank,
    then chunk 1.
"""

import sys

sys.path.insert(0, "/opt/trn_rl_repo")

import numpy as np

import concourse.bass as bass  # noqa: F401
import concourse.tile as tile
from concourse import bacc, mybir
from concourse.bass_utils import run_bass_kernel_spmd
from concourse.masks import make_identity

B = 8192
D = 1024
P = 128
NCORES = 8
R = B // NCORES  # 1024 rows per core
JBLK = 512  # j-block (keys/values) streamed per iteration
NH = R // JBLK  # 2 local row-halves (= gather chunks)
DO = D // P  # 8 feature chunks of 128
IC = R // P  # 8 row chunks of 128 per core
F32 = mybir.dt.float32
F32R = mybir.dt.float32r
BF16 = mybir.dt.bfloat16
F16 = mybir.dt.float16
AF = mybir.ActivationFunctionType
ALU = mybir.AluOpType
SCALE = 0.125  # 1/sqrt(head_dim=64)
RG = [list(range(NCORES))]


def build_program():
    nc = bacc.Bacc(
        "TRN2", target_bir_lowering=False, debug=False, num_devices=NCORES
    )
    # f32r is bit-identical to f32 (np.float32 feeds it); declaring the
    # matmul-bound inputs as f32r keeps their DMAs cast-free
    x_loc = nc.dram_tensor("x_loc", [R, D], F32R, kind="ExternalInput").ap()
    w_q = nc.dram_tensor("Wq", [D, D], F32R, kind="ExternalInput").ap()
    w_k = nc.dram_tensor("Wk", [D, D], F32R, kind="ExternalInput").ap()
    w_v = nc.dram_tensor("Wv", [D, D], F32R, kind="ExternalInput").ap()
    b_q = nc.dram_tensor("bq", [D], F32, kind="ExternalInput").ap()
    b_k = nc.dram_tensor("bk", [D], F32, kind="ExternalInput").ap()
    b_v = nc.dram_tensor("bv", [D], F32, kind="ExternalInput").ap()
    out_loc = nc.dram_tensor("out_loc", [R, D], F32, kind="ExternalOutput").ap()

    # local K^T/V shard chunks and their all-gathered forms (collective
    # in/out must be Internal DRAM; outputs Shared for the HBM-HBM path)
    kt_loc = nc.dram_tensor("kt_loc", [NH, DO, P, JBLK], F16)
    v_loc = nc.dram_tensor("v_loc", [NH, JBLK // P, P, D], BF16)
    kt_all = [
        nc.dram_tensor(f"kt_all{h}", [NCORES, DO, P, JBLK], F16, addr_space="Shared")
        for h in range(NH)
    ]
    v_all = [
        nc.dram_tensor(
            f"v_all{h}", [NCORES, JBLK // P, P, D], BF16, addr_space="Shared"
        )
        for h in range(NH)
    ]

    with tile.TileContext(nc) as tc:
        _body(
            nc, tc, x_loc, w_q, w_k, w_v, b_q, b_k, b_v, out_loc,
            kt_loc, v_loc, kt_all, v_all,
        )
    nc.compile()
    return nc


def _transpose_block(nc, tp_psum, identity, src_sb, dst, dd, col_off, drain_eng):
    """PE-transpose one [128, 128] block of src into dst[:, dd, col_off:+128],
    draining the PSUM through the given engine (scalar or vector)."""
    tp = tp_psum.tile([P, P], F32R, tag="tp")
    nc.tensor.transpose(tp, src_sb[:, dd * P : (dd + 1) * P], identity)
    if drain_eng == "scalar":
        nc.scalar.activation(dst[:, dd, col_off : col_off + P], tp, AF.Identity)
    else:
        nc.vector.tensor_copy(out=dst[:, dd, col_off : col_off + P], in_=tp)


def _body(nc, tc, x_loc, w_q, w_k, w_v, b_q, b_k, b_v, out_loc,
          kt_loc, v_loc, kt_all, v_all):
    from contextlib import ExitStack

    outer = ExitStack()
    outer.__enter__()
    # ---- persistent pools (whole kernel) ----
    const_pool = outer.enter_context(tc.tile_pool(name="const", bufs=1))
    identity_f32 = const_pool.tile([P, P], F32)
    make_identity(nc, identity_f32)
    identity = const_pool.tile([P, P], F32R)  # f32r for 1.5 cyc/row transposes
    nc.vector.tensor_copy(out=identity, in_=identity_f32)
    ones_f32 = const_pool.tile([P, 2], F32)
    nc.vector.memset(ones_f32, 1.0)
    ones = const_pool.tile([P, 2], BF16)
    nc.vector.tensor_copy(out=ones, in_=ones_f32)
    bq_sb = const_pool.tile([P, DO], F32)
    nc.sync.dma_start(bq_sb, b_q.rearrange("(oo p) -> p oo", p=P))
    bk_sb = const_pool.tile([P, DO], F32)
    nc.sync.dma_start(bk_sb, b_k.rearrange("(oo p) -> p oo", p=P))
    ones_row = const_pool.tile([1, P], F32)
    nc.vector.memset(ones_row, 1.0)
    # broadcast bv across all 128 partitions with a K=1 matmul
    bv_bc = const_pool.tile([P, D], F32)
    nc.sync.dma_start(bv_bc[0:1, :], b_v[None, :])
    with tc.tile_pool(name="bv_psum", bufs=2, space="PSUM") as bvp:
        for oh in range(2):
            pt = bvp.tile([P, 512], F32, tag="bvp")
            nc.tensor.matmul(
                pt,
                ones_row,
                bv_bc[0:1, oh * 512 : (oh + 1) * 512],
                start=True,
                stop=True,
            )
            nc.vector.tensor_copy(out=bv_bc[:, oh * 512 : (oh + 1) * 512], in_=pt)

    qt_pool = outer.enter_context(tc.tile_pool(name="qt", bufs=1))
    qt = qt_pool.tile([P, DO, R], F16)  # Q^T fp16: [o_in, o_out, i] (2 MB)

    sums_pool = outer.enter_context(tc.tile_pool(name="sums", bufs=1))
    sums_acc = sums_pool.tile([P, 2 * IC], F32)  # per-row exp-sums (even cols)
    rsum = sums_pool.tile([P, 2 * IC], F32)

    # =========================================================
    # Phase 1: weights+x^T up front, then K/V per row-half with
    # gathers fired K1,V1,K2,V2; Q^T last (overlaps the gathers)
    # =========================================================
    with ExitStack() as p1:
        wt_pool = p1.enter_context(tc.tile_pool(name="wt", bufs=1))
        wqt = wt_pool.tile([P, DO, D], F32R)  # W^T: [d_in, d_out, o] (4 MB)
        wkt = wt_pool.tile([P, DO, D], F32R)
        wvt = wt_pool.tile([P, DO, D], BF16)

        xt_pool = p1.enter_context(tc.tile_pool(name="xt", bufs=1))
        xt = xt_pool.tile([P, DO, R], F32R)  # x^T local (4 MB)
        xtb = xt_pool.tile([P, DO, R], BF16)  # bf16 copy for V matmul (2 MB)

        row_pool = p1.enter_context(tc.tile_pool(name="rows", bufs=3))
        st_pool = p1.enter_context(tc.tile_pool(name="stage", bufs=2))
        tp_psum = p1.enter_context(tc.tile_pool(name="tp_ps", bufs=2, space="PSUM"))
        mm_psum = p1.enter_context(tc.tile_pool(name="mm_ps", bufs=4, space="PSUM"))

        # -- all weight + x transposes first (their DMAs beat the AG traffic)
        for wt_sb, w_dram in ((wkt, w_k), (wvt, w_v), (wqt, w_q)):
            for oo in range(DO):
                wrow = row_pool.tile([P, D], F32R, tag="row")
                nc.sync.dma_start(wrow, w_dram[oo * P : (oo + 1) * P, :])
                for dd in range(DO):
                    _transpose_block(
                        nc, tp_psum, identity, wrow, wt_sb, dd, oo * P,
                        "scalar" if dd % 2 else "vector",
                    )
        for jj in range(IC):
            xrow = row_pool.tile([P, D], F32R, tag="row")
            nc.sync.dma_start(xrow, x_loc[jj * P : (jj + 1) * P, :])
            for dd in range(DO):
                _transpose_block(
                    nc, tp_psum, identity, xrow, xt, dd, jj * P,
                    "scalar" if dd % 2 else "vector",
                )
        nc.vector.tensor_copy(out=xtb, in_=xt)

        # -- per row-half: K^T shard then V shard, gathers K1,V1,K2,V2 --
        for ih in range(NH):
            for oo in range(DO):
                pk = mm_psum.tile([P, JBLK], F32, tag="mm")
                for dd in range(DO):
                    nc.tensor.matmul(
                        pk,
                        (wkt[:, dd, oo * P : (oo + 1) * P]),
                        (xt[:, dd, ih * JBLK : (ih + 1) * JBLK]),
                        start=(dd == 0),
                        stop=(dd == DO - 1),
                    )
                kst = st_pool.tile([P, JBLK], F16, tag="kst")
                nc.scalar.activation(
                    kst, pk, AF.Identity, bias=bk_sb[:, oo : oo + 1]
                )
                nc.sync.dma_start(kt_loc[ih, oo], kst)
            nc.gpsimd.collective_compute(
                "AllGather",
                mybir.AluOpType.bypass,
                replica_groups=RG,
                ins=[kt_loc[ih].opt()],
                outs=[kt_all[ih][:, :, :, :].opt()],
            )
            for jj in range(JBLK // P):
                vst = st_pool.tile([P, D], BF16, tag="vst")
                pv_h = [
                    mm_psum.tile([P, JBLK], F32, tag="mm", name="pv")
                    for _ in range(2)
                ]
                for dd in range(DO):
                    for oh in range(2):
                        nc.tensor.matmul(
                            pv_h[oh],
                            (xtb[:, dd, (ih * 4 + jj) * P : (ih * 4 + jj + 1) * P]),
                            (wvt[:, dd, oh * 512 : (oh + 1) * 512]),
                            start=(dd == 0),
                            stop=(dd == DO - 1),
                        )
                for oh in range(2):
                    nc.vector.tensor_copy(
                        out=vst[:, oh * 512 : (oh + 1) * 512], in_=pv_h[oh]
                    )
                nc.sync.dma_start(v_loc[ih, jj], vst)
            nc.gpsimd.collective_compute(
                "AllGather",
                mybir.AluOpType.bypass,
                replica_groups=RG,
                ins=[v_loc[ih].opt()],
                outs=[v_all[ih][:, :, :, :].opt()],
            )

        # -- Q^T (f32r, biased) — overlaps the in-flight gathers --
        for ih in range(NH):
            for oo in range(DO):
                pq = mm_psum.tile([P, JBLK], F32, tag="mm")
                for dd in range(DO):
                    nc.tensor.matmul(
                        pq,
                        (wqt[:, dd, oo * P : (oo + 1) * P]),
                        (xt[:, dd, ih * JBLK : (ih + 1) * JBLK]),
                        start=(dd == 0),
                        stop=(dd == DO - 1),
                    )
                nc.scalar.activation(
                    qt[:, oo, ih * JBLK : (ih + 1) * JBLK],
                    pq,
                    AF.Identity,
                    bias=bq_sb[:, oo : oo + 1],
                )

    # =========================================================
    # Phase 2: streamed attention in E^T layout, blocks walked in
    # gather-arrival order (chunk 0 of every rank, then chunk 1)
    # =========================================================
    with ExitStack() as p2:
        oa_pool = p2.enter_context(tc.tile_pool(name="oacc", bufs=1))
        outacc = oa_pool.tile([P, IC, D], F32)  # 4 MB

        kt_pool = p2.enter_context(tc.tile_pool(name="ktb", bufs=3))
        v_pool = p2.enter_context(tc.tile_pool(name="vtb", bufs=3))
        pt_pool = p2.enter_context(tc.tile_pool(name="ptb", bufs=3))
        e_psum = p2.enter_context(tc.tile_pool(name="e_ps", bufs=4, space="PSUM"))
        o_psum = p2.enter_context(tc.tile_pool(name="o_ps", bufs=3, space="PSUM"))
        s_psum = p2.enter_context(tc.tile_pool(name="s_ps", bufs=1, space="PSUM"))

        first = True
        for ch in range(NH):
            for g in range(NCORES):
                ktb = kt_pool.tile([P, DO, JBLK], F16, tag="ktb")
                for oo in range(DO):
                    nc.sync.dma_start(ktb[:, oo, :], kt_all[ch][g, oo])
                vtb = v_pool.tile([P, JBLK // P, D], BF16, tag="vtb")
                nc.sync.dma_start(
                    vtb, v_all[ch][g].rearrange("jj p o -> p jj o")
                )
                # unnormalized probabilities P^T for this j-block: [j, i]
                ptb = pt_pool.tile([P, JBLK // P, R], BF16, tag="ptb")
                for jj in range(JBLK // P):
                    pe_h = [
                        e_psum.tile([P, JBLK], F32, tag="pe", name="pe")
                        for _ in range(R // JBLK)
                    ]
                    for oo in range(DO):
                        for ih in range(R // JBLK):
                            nc.tensor.matmul(
                                pe_h[ih],
                                (ktb[:, oo, jj * P : (jj + 1) * P]),
                                (qt[:, oo, ih * JBLK : (ih + 1) * JBLK]),
                                start=(oo == 0),
                                stop=(oo == DO - 1),
                            )
                    for ih in range(R // JBLK):
                        nc.scalar.activation(
                            ptb[:, jj, ih * JBLK : (ih + 1) * JBLK],
                            pe_h[ih],
                            AF.Exp,
                            scale=SCALE,
                        )
                # out_unnorm += P^T.T @ V; exp-sums matmul shares each
                # stationary ptb tile (3 streams per weight load)
                ps = s_psum.tile([P, 2 * IC], F32, tag="ps")
                for ic in range(IC):
                    po_h = [
                        o_psum.tile([P, 512], F32, tag="po", name="po")
                        for _ in range(2)
                    ]
                    for jj in range(JBLK // P):
                        for oh in range(2):
                            nc.tensor.matmul(
                                po_h[oh],
                                (ptb[:, jj, ic * P : (ic + 1) * P]),
                                (vtb[:, jj, oh * 512 : (oh + 1) * 512]),
                                start=(jj == 0),
                                stop=(jj == JBLK // P - 1),
                            )
                        nc.tensor.matmul(
                            ps[:, 2 * ic : 2 * ic + 2],
                            (ptb[:, jj, ic * P : (ic + 1) * P]),
                            (ones),
                            start=(ic == 0 and jj == 0),
                            stop=(ic == IC - 1 and jj == JBLK // P - 1),
                        )
                    for oh in range(2):
                        dst = outacc[:, ic, oh * 512 : (oh + 1) * 512]
                        if first:
                            nc.vector.tensor_copy(out=dst, in_=po_h[oh])
                        else:
                            nc.vector.tensor_tensor(dst, po_h[oh], dst, ALU.add)
                if first:
                    nc.vector.tensor_copy(out=sums_acc, in_=ps)
                else:
                    nc.vector.tensor_tensor(sums_acc, ps, sums_acc, ALU.add)
                first = False

        # ---- epilogue: normalize, add bv, write out ----
        nc.vector.reciprocal(rsum, sums_acc)
        fin_pool = p2.enter_context(tc.tile_pool(name="fin", bufs=2))
        for ic in range(IC):
            ofin = fin_pool.tile([P, D], F32, tag="ofin")
            nc.vector.tensor_scalar_mul(ofin, outacc[:, ic, :], rsum[:, 2 * ic : 2 * ic + 1])
            nc.vector.tensor_tensor(ofin, ofin, bv_bc, ALU.add)
            nc.sync.dma_start(out_loc[ic * P : (ic + 1) * P, :], ofin)

    outer.close()


_NC_CACHE = None


def _get_program():
    global _NC_CACHE
    if _NC_CACHE is None:
        _NC_CACHE = build_program()
    return _NC_CACHE


def _run(inputs, trace=False):
    nc = _get_program()
    x = np.ascontiguousarray(np.asarray(inputs["x"], dtype=np.float32))
    common = {
        k: np.ascontiguousarray(np.asarray(inputs[k], dtype=np.float32))
        for k in ("Wq", "Wk", "Wv", "bq", "bk", "bv")
    }
    in_maps = [
        {"x_loc": np.ascontiguousarray(x[c * R : (c + 1) * R]), **common}
        for c in range(NCORES)
    ]
    res = run_bass_kernel_spmd(
        nc, in_maps, core_ids=list(range(NCORES)), trace=trace
    )
    out = np.concatenate([res.results[c]["out_loc"] for c in range(NCORES)], axis=0)
    return out.reshape(B, D, 1).astype(np.float32), res


def kernel(**inputs):
    out, _ = _run(inputs, trace=False)
    return out
